# revision 20
# baseline (speedup 1.0000x reference)
"""Trainium2 Bass kernel for causal self-attention with segment masking.

Sharding: 8 cores = 2 batches x 4 head-groups (4 heads each).
Per core: QKV projection (bf16), S^T-layout attention with data-dependent
tile skipping AND per-tile q-column windowing (causal + segment structure),
output projection producing a partial [T, D] sum; host adds the 4 partials
per batch.

Layouts (per core):
  x_sb  [128, 4qc, 8i, 512] bf16 (host pre-tiled so each input DMA moves
        8KB contiguous per partition; x[d=i*128+p, t=qc*512+u])
  q_T/k_T [128, T]  bf16  two tiles, one per head pair (2 heads x 64 dims)
  v_ext [128, 16kb, 4h, 65] bf16 (col 64 = ones -> softmax denominator)
  s     [128k, 2h, 512q] f32 PSUM, written only on the tile's live window
  pt    [128, 2, 512] bf16 SBUF = exp(s/8) * mask01 (window only)
  y_ps  [65, 512]   f32 PSUM = v_ext.T @ pt (row 64 = sum of p = denom);
        ragged window accumulation relies on per-element has_written bits
  y_qc  [128, 2, T] bf16 (normalized, feeds proj as lhsT)

Scheduling notes (the performance-critical part):
  - Every engine queue executes strictly in order, so the softmax
    normalization chain (y PSUM->SBUF evac, denom-row gather DMA, DVE
    reciprocal, gather-back DMA, gpsimd broadcast DMA, scaling multiplies)
    is emitted in pipeline stages spread over the next two attention
    groups; each op is emitted at least one full group after its producer.
  - 1/denom is broadcast over partitions with gpsimd partition-replicating
    DMAs into an SBUF tile (keeps the PE free of rank-1 broadcast
    matmuls); head 1's unnormalized y is partition-shifted to 64-127 a
    full group early so the final multiplies write y_qc in place.
  - A short junk-matmul burn after the first weight-chunk DMA opens the
    HAM clock gate (PE idles at 1.2 GHz otherwise) before real work.
  - PSUM budget (8 banks): psq 2 (qkv/proj chains) + pss 4 (score tiles)
    + psy 2 (y tiles).
"""

import numpy as np
import ml_dtypes

import concourse.bass as bass
import concourse.mybir as mybir
import concourse.tile as tile
from concourse import bacc
from concourse import bass_utils

B, T, D = 2, 2048, 1024
H, HD = 16, 64
QC = 512            # q chunk (max matmul free dim)
KB = 128            # k block (partition dim)
NQC = T // QC       # 4
NKB = T // KB       # 16
DK = D // 128       # 8 contraction chunks for projections
BF16 = mybir.dt.bfloat16
F32 = mybir.dt.float32
nbf = ml_dtypes.bfloat16
Exp = mybir.ActivationFunctionType.Exp


def _schedule(seg):
    """Data-dependent tile schedule, shared (union) across both batches.

    Returns (act, mask_arrs, wtot):
      act: {qc: [(kb, w0, w1, moff)]} where [w0, w1) is the live q-column
           window within the chunk and moff the column offset of this
           tile's mask in the packed mask tensor (-1 = all-ones window).
      mask_arrs: per-batch packed bf16 {0,1} masks [KB, wtot].
    """
    ar = np.arange(T)
    masks = [
        (seg[b][:, None] == seg[b][None, :]) & (ar[:, None] <= ar[None, :])
        for b in range(B)
    ]  # mask_T[k, q]
    act = {qc: [] for qc in range(NQC)}
    mask_cols = [[] for _ in range(B)]
    wtot = 0
    for qc in range(NQC):
        for kb in range(NKB):
            if kb * KB > qc * QC + QC - 1:
                continue  # fully above the diagonal
            subs = [
                masks[b][kb * KB:(kb + 1) * KB, qc * QC:(qc + 1) * QC]
                for b in range(B)
            ]
            u = subs[0] | subs[1]
            if not u.any():
                continue  # dead tile in both batches: skip entirely
            idx = np.nonzero(u.any(axis=0))[0]
            w0 = int(idx[0]) & ~3
            w1 = min(QC, (int(idx[-1]) + 4) & ~3)
            win = [s[:, w0:w1] for s in subs]
            if all(w.all() for w in win):
                act[qc].append((kb, w0, w1, -1))
            else:
                act[qc].append((kb, w0, w1, wtot))
                for b in range(B):
                    mask_cols[b].append(win[b].astype(nbf))
                wtot += w1 - w0
    if wtot == 0:
        wtot = 4
        mask_arrs = [np.zeros((KB, 4), nbf) for _ in range(B)]
    else:
        mask_arrs = [
            np.ascontiguousarray(np.concatenate(mask_cols[b], axis=1))
            for b in range(B)
        ]
    return act, mask_arrs, wtot


def _build(act, wtot):
    nc = bacc.Bacc("TRN2", target_bir_lowering=False, debug=False, num_devices=8)
    xq = nc.dram_tensor("xq", [NQC, 128, DK, QC], BF16, kind="ExternalInput").ap()
    # qkv weights split by destination (q/k/v) so the startup-critical q
    # columns can land first: w3[j, p, i, n] = W[i*128+p, j*256+n]
    w3 = nc.dram_tensor("w3", [3, 128, DK, 256], BF16, kind="ExternalInput").ap()
    wp = nc.dram_tensor("wp", [128, 2, D], BF16, kind="ExternalInput").ap()
    mk = nc.dram_tensor("mask", [KB, wtot], BF16, kind="ExternalInput").ap()
    ind = nc.dram_tensor("ind", [2, 128], BF16, kind="ExternalInput").ap()
    out = nc.dram_tensor("out", [T, D], BF16, kind="ExternalOutput").ap()

    with tile.TileContext(nc) as tc:
        with (
            tc.tile_pool(name="const", bufs=1) as cpool,
            tc.tile_pool(name="ptp", bufs=4) as ppool,
            tc.tile_pool(name="otp", bufs=4) as opool,
            tc.tile_pool(name="nrm", bufs=4) as npool,
            tc.tile_pool(name="psq", bufs=2, space="PSUM") as psq,
            tc.tile_pool(name="pss", bufs=2, space="PSUM") as pss,
            tc.tile_pool(name="psy", bufs=1, space="PSUM") as psy,
        ):
            # constant junk tile: lets PE warm-up matmuls start before any
            # DMA lands (memset has no dependencies)
            junk_sb = cpool.tile([128, 512], BF16, tag="junk")
            nc.vector.memset(junk_sb[:], 1.0)

            # ---- input DMAs (sync HWDGE queue; all transfers are >=2KB
            # contiguous per partition thanks to the host pre-tiling).
            # Priority: q weights, x[qc0], k/v weights, mask, x[qc1..3])
            w_sb = cpool.tile([128, 3, DK, 256], BF16, tag="w3")
            x_sb = cpool.tile([128, NQC, DK, QC], BF16, tag="x")
            nc.scalar.dma_start(w_sb[:, 0], w3[0])
            nc.sync.dma_start(x_sb[:, 0, 0:4], xq[0][:, 0:4])
            nc.scalar.dma_start(x_sb[:, 0, 4:DK], xq[0][:, 4:DK])
            nc.scalar.dma_start(w_sb[:, 1], w3[1])
            nc.scalar.dma_start(w_sb[:, 2], w3[2])
            # masks for the first q-chunk's tiles (a prefix of the packed
            # mask tensor) land before x[qc1..3] so attn(0,*) never waits
            msplit = max((m + (w1 - w0) for (_, w0, w1, m) in act[0] if m >= 0), default=0)
            mask_sb = cpool.tile([128, wtot], BF16, tag="m")
            if msplit > 0:
                nc.sync.dma_start(mask_sb[:, 0:msplit], mk[:, 0:msplit])
            def x_chunk(qc):
                nc.sync.dma_start(x_sb[:, qc], xq[qc])
            x_chunk(1)
            if msplit < wtot:
                nc.sync.dma_start(mask_sb[:, msplit:wtot], mk[:, msplit:wtot])
            x_chunk(2)
            wp_sb = cpool.tile([128, 2, D], BF16, tag="wp")
            nc.scalar.dma_start(wp_sb[:], wp[:])
            x_chunk(3)

            q_sb = [cpool.tile([128, T], BF16, tag=f"q{p}", name=f"q{p}") for p in range(2)]
            k_sb = [cpool.tile([128, T], BF16, tag=f"k{p}", name=f"k{p}") for p in range(2)]
            v_sb = cpool.tile([128, NKB, 4, 65], BF16, tag="v")
            y_qc = [cpool.tile([128, 2, QC], BF16, tag=f"y{qc}", name=f"y{qc}") for qc in range(NQC)]
            nc.vector.memset(v_sb[:, :, :, 64], 1.0)
            # block indicator for the 1/denom partition broadcast: one K=2
            # matmul maps l0 row 0 -> partitions 0-63, row 1 -> 64-127
            ind_sb = cpool.tile([2, 128], BF16, tag="ind")
            nc.sync.dma_start(ind_sb[:], ind[:])
            # trigger the exp table-set load (~2.7us) before attention needs it
            tw = npool.tile([1, 64], BF16, tag="tw")
            nc.scalar.activation(tw[:], junk_sb[0:1, 0:64], Exp, scale=1.0)

            # PE warm-up / keep-warm burn: dependency-free junk matmuls.
            # At kernel start they open the HAM clock-gate while input DMAs
            # land; at the tail they bridge sub-2us gaps so the PE clock
            # never re-throttles to 1.2 GHz.
            _burn_n = [0]

            def emit_burn(n):
                _burn_n[0] += 1
                bt = psq.tile([128, 512], F32, tag="psq", name=f"burn{_burn_n[0]}")
                for _ in range(n):
                    nc.tensor.matmul(
                        bt[:], junk_sb[:, 0:128], junk_sb[:],
                        start=True, stop=True,
                    )

            emit_burn(27)

            # ---- building blocks ----
            def emit_qkv_qk(qc):
                for p in range(2):
                    ps = psq.tile([128, 512], F32, tag="psq", name=f"q_{qc}_{p}")
                    for i in range(DK):
                        nc.tensor.matmul(
                            ps[:], w_sb[:, 0, i, p * 128:(p + 1) * 128],
                            x_sb[:, qc, i, :],
                            start=(i == 0), stop=(i == DK - 1),
                        )
                    nc.vector.tensor_copy(out=q_sb[p][:, qc * 512:(qc + 1) * 512], in_=ps[:])
                for p in range(2):
                    ps = psq.tile([128, 512], F32, tag="psq", name=f"k_{qc}_{p}")
                    for i in range(DK):
                        nc.tensor.matmul(
                            ps[:], w_sb[:, 1, i, p * 128:(p + 1) * 128],
                            x_sb[:, qc, i, :],
                            start=(i == 0), stop=(i == DK - 1),
                        )
                    nc.vector.tensor_copy(out=k_sb[p][:, qc * 512:(qc + 1) * 512], in_=ps[:])

            def emit_qkv_v(qc):
                for u in v_units(qc):
                    u()

            def emit_attn(qc, p, fill=None, mask_eng=None, last=False):
                # fill: list of zero-arg emitters (qkv/proj PSUM-group units)
                # woven between attention tiles, after each tile's QK and
                # before its PV, so the PE stays busy (and the HAM clock
                # stays at 2.4 GHz) while exp/mask latency plays out.
                fill = list(fill) if fill else []
                mask_eng = mask_eng or nc.vector
                kbs = act[qc]
                y_ps = psy.tile([128, 2, 512], F32, tag="psy", name=f"yps{p}_{qc}")
                for idx, (kb, w0, w1, moff) in enumerate(kbs):
                    w = w1 - w0
                    first, lastt = idx == 0, idx == len(kbs) - 1
                    s_ps = pss.tile([128, 2, 512], F32, tag="pss", name=f"s_{p}_{qc}_{kb}")
                    for hh in range(2):
                        lo = hh * 64
                        nc.tensor.matmul(
                            s_ps[:, hh, w0:w1],
                            k_sb[p][lo:lo + 64, kb * 128:(kb + 1) * 128],
                            q_sb[p][lo:lo + 64, qc * 512 + w0:qc * 512 + w1],
                            start=True, stop=True,
                        )
                    pt = ppool.tile([128, 2, 512], BF16, tag="pt", name=f"pt{p}_{qc}_{kb}")
                    nc.scalar.activation(pt[:, :, w0:w1], s_ps[:, :, w0:w1], Exp, scale=0.125)
                    if moff >= 0:
                        mask_eng.tensor_tensor(
                            out=pt[:, :, w0:w1],
                            in0=pt[:, :, w0:w1],
                            in1=mask_sb[:, None, moff:moff + w].to_broadcast((128, 2, w)),
                            op=mybir.AluOpType.mult,
                        )
                    if fill:
                        fill.pop(0)()
                    for hh in range(2):
                        nc.tensor.matmul(
                            y_ps[0:65, hh, w0:w1], v_sb[:, kb, p * 2 + hh, :],
                            pt[:, hh, w0:w1],
                            start=first, stop=lastt, skip_group_check=True,
                        )
                # evacuate y to SBUF fast (releases the PSUM banks); the
                # normalization chain is emitted in stages spread over the
                # next two groups so no queue ever stalls mid-chain.
                y_sb = npool.tile([65, 2, 512], BF16, tag="ysb", name=f"ysb{qc}_{p}")
                if last:
                    # final group: evacuate the denominator row first (so the
                    # lp gather can start immediately) and split the body
                    # copies across scalar+vector to shorten the tail chain
                    nc.scalar.copy(out=y_sb[64:65, :, :], in_=y_ps[64:65, :, :])
                    nc.vector.tensor_copy(out=y_sb[0:64, 0, :], in_=y_ps[0:64, 0, :])
                    nc.scalar.copy(out=y_sb[0:64, 1, :], in_=y_ps[0:64, 1, :])
                else:
                    nc.scalar.copy(out=y_sb[:], in_=y_ps[0:65, :, :])
                for u in fill:
                    u()
                return {"qc": qc, "p": p, "y_sb": y_sb}

            # normalization pipeline stages (row 64 of y_sb = denominators)
            def emit_lp(n):
                n["lp"] = npool.tile([128, 8], BF16, tag="lp", name=f"lp{n['qc']}_{n['p']}")
                nc.sync.dma_start(n["lp"][:], n["y_sb"][64:65, :, :])
                # gather both heads' unnormalized y into one [128,512] tile
                # (head 1 partition-shifted to 64-127) a full group before
                # the multiply needs it
                n["ysh"] = npool.tile([128, 512], BF16, tag="ysh", name=f"ysh{n['qc']}_{n['p']}")
                nc.sync.dma_start(n["ysh"][0:64, :], n["y_sb"][0:64, 0, :])
                nc.sync.dma_start(n["ysh"][64:128, :], n["y_sb"][0:64, 1, :])

            def emit_recip(n):
                lpb = npool.tile([128, 8], BF16, tag="lpb", name=f"lpb{n['qc']}_{n['p']}")
                with nc.allow_low_precision(reason="bf16 softmax denominators"):
                    nc.vector.reciprocal(lpb[:], n["lp"][:])
                # l0 [2, 512]: row hh = 1/den for head hh, in q order
                n["l0"] = npool.tile([2, 512], BF16, tag="l0", name=f"l0{n['qc']}_{n['p']}")
                nc.sync.dma_start(n["l0"][0:1, :], lpb[0:64, :])
                nc.sync.dma_start(n["l0"][1:2, :], lpb[64:128, :])

            def emit_finish(n):
                # broadcast 1/denom over the partitions with a single K=2
                # indicator matmul (l0 row hh -> partition block hh), then
                # scale y straight into the projection's lhsT layout; head 0
                # reads y_sb in place, head 1 reads the partition-shifted copy
                qc, p = n["qc"], n["p"]
                lb = psq.tile([128, 512], F32, tag="psq", name=f"lb{qc}_{p}")
                nc.tensor.matmul(lb[:], ind_sb[:], n["l0"][:], start=True, stop=True)
                nc.vector.tensor_mul(
                    out=y_qc[qc][:, p, :], in0=n["ysh"][:], in1=lb[:])

            # ---- filler units: single-PSUM-group emitters for weaving into
            # attention groups (each is an independent chunk of PE work) ----
            def qk_units(qc):
                units = []
                for which, j, dsts in (("q", 0, q_sb), ("k", 1, k_sb)):
                    for p in range(2):
                        def u(qc=qc, p=p, j=j, which=which, dsts=dsts):
                            ps = psq.tile([128, 512], F32, tag="psq", name=f"{which}_{qc}_{p}")
                            for i in range(DK):
                                nc.tensor.matmul(
                                    ps[:], w_sb[:, j, i, p * 128:(p + 1) * 128],
                                    x_sb[:, qc, i, :],
                                    start=(i == 0), stop=(i == DK - 1),
                                )
                            nc.vector.tensor_copy(
                                out=dsts[p][:, qc * 512:(qc + 1) * 512], in_=ps[:])
                        units.append(u)
                return units

            def v_units(qc):
                # two k-blocks share one [128,512] psum (col halves) and a
                # single evacuation copy
                units = []
                for kb in range(qc * 4, qc * 4 + 4, 2):
                    def u(qc=qc, kb=kb):
                        ps = psq.tile([128, 512], F32, tag="psq", name=f"v_{kb}")
                        for h in range(2):
                            for i in range(DK):
                                nc.tensor.matmul(
                                    ps[:, h * 256:(h + 1) * 256],
                                    x_sb[:, qc, i, (kb % 4 + h) * 128:(kb % 4 + h) * 128 + 128],
                                    w_sb[:, 2, i, :],
                                    start=(i == 0), stop=(i == DK - 1),
                                    skip_group_check=True,
                                )
                        nc.vector.tensor_copy(
                            out=v_sb[:, kb:kb + 2, :, 0:64],
                            in_=ps[:].rearrange("p (b h d) -> p b h d", b=2, h=4),
                        )
                    units.append(u)
                return units

            _ots = {}

            def proj_units(qc, out_eng=None):
                # two units per output tile (one per 512-col half); the out
                # DMA rides the given queue after the second half's copy
                out_eng = out_eng or nc.gpsimd
                units = []
                for mt in range(qc * 4, qc * 4 + 4):
                    def u0(qc=qc, mt=mt):
                        ot = opool.tile([128, 1024], BF16, tag="ot", name=f"ot{mt}")
                        _ots[mt] = ot
                        ps = psq.tile([128, 512], F32, tag="psq", name=f"pso{mt}_0")
                        for c in range(2):
                            nc.tensor.matmul(
                                ps[:], y_qc[qc][:, c, (mt % 4) * 128:(mt % 4) * 128 + 128],
                                wp_sb[:, c, 0:512],
                                start=(c == 0), stop=(c == 1),
                            )
                        nc.vector.tensor_copy(out=ot[:, 0:512], in_=ps[:])
                        out_eng.dma_start(
                            out[mt * 128:(mt + 1) * 128, 0:512], ot[:, 0:512])
                    def u1(qc=qc, mt=mt, out_eng=out_eng):
                        ot = _ots[mt]
                        ps = psq.tile([128, 512], F32, tag="psq", name=f"pso{mt}_1")
                        for c in range(2):
                            nc.tensor.matmul(
                                ps[:], y_qc[qc][:, c, (mt % 4) * 128:(mt % 4) * 128 + 128],
                                wp_sb[:, c, 512:1024],
                                start=(c == 0), stop=(c == 1),
                            )
                        nc.scalar.copy(out=ot[:, 512:1024], in_=ps[:])
                        out_eng.dma_start(
                            out[mt * 128:(mt + 1) * 128, 512:1024], ot[:, 512:1024])
                    units += [u0, u1]
                return units

            def emit_proj(qc, out_eng=None):
                for u in proj_units(qc, out_eng=out_eng):
                    u()

            def emit_proj_tail(qc):
                # attention is done: use the idle score-PSUM pool for
                # whole-tile psums (third rotation slot keeps the PE
                # cadence matmul-bound), one wide copy per tile on
                # alternating engines, full-row output DMA
                for j, mt in enumerate(range(qc * 4, qc * 4 + 4)):
                    ot = opool.tile([128, 1024], BF16, tag="ot", name=f"ot{mt}")
                    pst = pss.tile([128, 2, 512], F32, tag="pss", name=f"pso{mt}")
                    for n in range(2):
                        for c in range(2):
                            nc.tensor.matmul(
                                pst[:, n, :],
                                y_qc[qc][:, c, (mt % 4) * 128:(mt % 4) * 128 + 128],
                                wp_sb[:, c, n * 512:(n + 1) * 512],
                                start=(c == 0), stop=(c == 1),
                            )
                    if j % 2 == 0:
                        nc.vector.tensor_copy(out=ot[:], in_=pst[:])
                    else:
                        nc.scalar.copy(out=ot[:], in_=pst[:])
                    nc.gpsimd.dma_start(out[mt * 128:(mt + 1) * 128, :], ot[:])

            # tail c-split for the last output block: the first contraction
            # half (head pair 0) is computed as soon as finish(qc3, p0) is
            # done; only the second half trails the last normalization
            def proj3_c0():
                _ots["ps12"] = pss.tile([128, 2, 512], F32, tag="pss", name="pso12")
                _ots["ps13"] = pss.tile([128, 2, 512], F32, tag="pss", name="pso13")
                ps14 = psy.tile([128, 2, 512], F32, tag="psy", name="pso14")
                _ots["ps14"] = [ps14[:, 0, :], ps14[:, 1, :]]
                for mt in (12, 13, 14):
                    for n in range(2):
                        ps = _ots[f"ps{mt}"][:, n, :] if mt < 14 else _ots["ps14"][n]
                        nc.tensor.matmul(
                            ps, y_qc[3][:, 0, (mt % 4) * 128:(mt % 4) * 128 + 128],
                            wp_sb[:, 0, n * 512:(n + 1) * 512],
                            start=True, stop=False, skip_group_check=True,
                        )

            def proj3_c1():
                engs = [nc.sync, nc.gpsimd, nc.sync]
                for mt, eng in zip((12, 13, 14), engs):
                    ot = opool.tile([128, 1024], BF16, tag="ot", name=f"ot{mt}")
                    for n in range(2):
                        ps = _ots[f"ps{mt}"][:, n, :] if mt < 14 else _ots["ps14"][n]
                        nc.tensor.matmul(
                            ps, y_qc[3][:, 1, (mt % 4) * 128:(mt % 4) * 128 + 128],
                            wp_sb[:, 1, n * 512:(n + 1) * 512],
                            start=False, stop=True, skip_group_check=True,
                        )
                        if n == 0:
                            nc.vector.tensor_copy(out=ot[:, 0:512], in_=ps)
                        else:
                            nc.scalar.copy(out=ot[:, 512:1024], in_=ps)
                    eng.dma_start(out[mt * 128:(mt + 1) * 128, :], ot[:])
                # final tile computed whole (psum slots freed by now); each
                # half ships the moment its copy lands
                mt = 15
                ot = opool.tile([128, 1024], BF16, tag="ot", name="ot15")
                pst = pss.tile([128, 2, 512], F32, tag="pss", name="pso15")
                for n in range(2):
                    ps = pst[:, n, :]
                    for c in range(2):
                        nc.tensor.matmul(
                            ps, y_qc[3][:, c, (mt % 4) * 128:(mt % 4) * 128 + 128],
                            wp_sb[:, c, n * 512:(n + 1) * 512],
                            start=(c == 0), stop=(c == 1),
                        )
                    if n == 0:
                        nc.vector.tensor_copy(out=ot[:, 0:512], in_=ps)
                        nc.gpsimd.dma_start(
                            out[mt * 128:(mt + 1) * 128, 0:512], ot[:, 0:512])
                    else:
                        nc.scalar.copy(out=ot[:, 512:1024], in_=ps)
                        nc.sync.dma_start(
                            out[mt * 128:(mt + 1) * 128, 512:1024], ot[:, 512:1024])

            # ---- schedule ----
            # Normalization of group g is pipelined across the next two
            # attention groups: lp-dma before T(g+1); recip+l0 after
            # T(g+1); broadcast matmul + scaling multiplies after T(g+2).
            # Every op is emitted at least a full group after its producer,
            # so no engine queue FIFO-blocks mid-chain. From T(2,0) on, the
            # remaining qkv chunks and all proj work ride inside the
            # attention groups as fillers; masks of those groups move to
            # gpsimd so the DVE queue stays clear for evacs and finishes.
            emit_qkv_qk(0)
            emit_qkv_v(0)
            n0 = emit_attn(0, 0)
            emit_qkv_qk(1)
            emit_lp(n0)
            n1 = emit_attn(0, 1)
            emit_recip(n0)
            emit_qkv_v(1)
            emit_lp(n1)
            n2 = emit_attn(1, 0)
            emit_recip(n1)
            emit_finish(n0)
            emit_qkv_qk(2)
            emit_lp(n2)
            n3 = emit_attn(1, 1)
            emit_recip(n2)
            emit_finish(n1)
            emit_qkv_v(2)
            emit_lp(n3)
            n4 = emit_attn(2, 0, fill=qk_units(3))
            emit_recip(n3)
            emit_finish(n2)
            emit_lp(n4)
            n5 = emit_attn(2, 1, fill=v_units(3))
            emit_recip(n4)
            emit_finish(n3)
            emit_lp(n5)
            n6 = emit_attn(3, 0, fill=proj_units(0))
            emit_recip(n5)
            emit_finish(n4)
            emit_lp(n6)
            n7 = emit_attn(
                3, 1,
                fill=proj_units(1) + [lambda: emit_finish(n5)],
                last=True)
            emit_lp(n7)
            emit_recip(n6)
            emit_proj_tail(2)
            emit_finish(n6)
            proj3_c0()
            emit_recip(n7)
            emit_finish(n7)
            proj3_c1()

    nc.compile()
    return nc


def _in_maps(x, seg, Wqkv, Wproj, mask_arrs):
    maps = []
    for c in range(8):
        b, g = divmod(c, 4)
        h0 = g * 4
        cs, ce = h0 * 64, h0 * 64 + 256
        xT = np.ascontiguousarray(x[b].T).astype(nbf)
        wcat = np.concatenate(
            [Wqkv[:, cs:ce], Wqkv[:, D + cs:D + ce], Wqkv[:, 2 * D + cs:2 * D + ce]],
            axis=1)
        maps.append({
            # [NQC, 128, DK, QC]: xq[qc, p, i, u] = xT[i*128+p, qc*512+u]
            "xq": np.ascontiguousarray(
                xT.reshape(DK, 128, NQC, QC).transpose(2, 1, 0, 3)),
            # [3, 128, DK, 256]: w3[j, p, i, n] = wcat[i*128+p, j*256+n]
            "w3": np.ascontiguousarray(
                wcat.reshape(DK, 128, 3, 256).transpose(2, 1, 0, 3).astype(nbf)),
            # [128, 2, D]: wp[p, c, n] = Wproj[cs + c*128 + p, n]
            "wp": np.ascontiguousarray(
                Wproj[cs:ce, :].reshape(2, 128, D).transpose(1, 0, 2).astype(nbf)),
            "mask": mask_arrs[b],
            "ind": _IND,
        })
    return maps


# block indicator for the 1/denom broadcast matmul
_IND = np.zeros((2, 128), nbf)
_IND[0, 0:64] = 1
_IND[1, 64:128] = 1


_CACHE = {}


def _prepare(x, segment_ids, W_qkv, W_proj):
    x = np.asarray(x, np.float32)
    seg = np.asarray(segment_ids)
    Wqkv = np.asarray(W_qkv, np.float32)
    Wproj = np.asarray(W_proj, np.float32)
    tiles, mask_arrs, wtot = _schedule(seg)
    key = (tuple((qc, t) for qc in tiles for t in tiles[qc]), wtot)
    if key not in _CACHE:
        _CACHE[key] = _build(tiles, wtot)
    nc = _CACHE[key]
    return nc, _in_maps(x, seg, Wqkv, Wproj, mask_arrs)


def kernel(x, segment_ids, W_qkv, W_proj):
    nc, in_maps = _prepare(x, segment_ids, W_qkv, W_proj)
    res = bass_utils.run_bass_kernel_spmd(nc, in_maps, core_ids=list(range(8)))
    out = np.zeros((B, T, D), np.float32)
    for c in range(8):
        out[c // 4] += res.results[c]["out"].astype(np.float32)
    return out
